# revision 22
# baseline (speedup 1.0000x reference)
"""Depthwise causal conv1d kernel for Trainium2 (8 NeuronCores, SPMD).

Problem: x [B=8, T=4096, C=512] f32, weight [C=512, K=4] f32.
out[b, t, c] = sum_k weight[c, k] * x[b, t - 3 + k, c]   (causal, zero-pad)

Strategy:
  - Data-parallel over batch: core b handles x[b].
  - Host-side layout: each core's input is channels-first x[b].T padded
    with K-1 = 3 leading zeros along time -> [C=512, T+3=4099], cast to
    fp16 (halves HBM traffic; ~2^-11 element error), accumulation fp32.
  - Device: TensorE computes only taps 0..2 as accumulating diag-matmuls
    (3 matmuls per 512-wide half, PSUM pair-tiles of [128,1024]); the
    4th tap is fused into the PSUM->SBUF drain on the Vector engine:
      out_f16 = (x_shift3 * w3) + psum        (scalar_tensor_tensor)
    which costs the same as the plain cast-copy it replaces, cutting PE
    work 25% (128 -> 96 matmuls, ~27.5us -> ~20.7us of PE stream).
  - ~10 warmup matmuls on scratch SBUF pre-ramp the PE p-state during
    the otherwise-idle wait for the first x DMA, so real matmuls issue
    at the full 0.42 ns/row rate from the start.
  - Input DMAs are triggered from the Scalar engine's DGE queue (ACT is
    otherwise idle in this design) so they issue in parallel with the
    output DMAs on Sync and ahead of Sync's slower preamble; outputs
    ship per half-chunk (last chunk per pair) to shorten the tail.
  - The kernel is DMA-bound: 8.4 MB of HBM traffic per core across 16
    DMA engines (~22.5 B/ns each) ~= 23.3 us, with PE (~20.7us) and
    DVE (~19.1us) hidden underneath.
"""

import numpy as np

B, T, C, K = 8, 4096, 512, 4
P = 128  # partitions
NCHUNK = C // P  # 4 channel chunks
TJ = 512  # time-tile (free dim) per matmul; one PSUM bank
NJ = T // TJ  # 8 time tiles per chunk
TP = T + K - 1  # padded time = 4099
NW = NCHUNK * K  # 16 (chunk, tap) columns in the weight table
KM = K - 1  # taps done on TensorE (0..2); tap 3 fused on DVE
NWARM = 30  # PE p-state warmup matmuls: 128-wide (~110-150ns each at
# mid p-state), bridging the PE from ~7us (gpsimd memset + sem) to the
# first x piece's semaphore ~10.2-10.7us with fine granularity, so the
# PE never idles — an idle gap resets the ~4.2us DVFS ramp. Groups of 3
# accumulating matmuls alternating PSUM halves — standalone start&stop
# matmuls on one bank serialize on the PSUM write port and never speed
# up. Overshooting by one warmup costs ~150ns; a gap costs ~2us.
GP_OFF = ()  # GpSimd pair-tile offload is dead: neuronxcc rejects
# TensorScalarPtr on the Pool engine ("Instruction engine check failed")

_compiled = None


def _build():
    import concourse.bacc as bacc
    import concourse.bass as bass
    import concourse.mybir as mybir
    from concourse.tile import TileContext

    f32 = mybir.dt.float32
    f16 = mybir.dt.float16
    nc = bacc.Bacc(enable_partition_id=False)

    wt_d = nc.declare_dram_parameter("wt", [P, NW], f16, isOutput=False)
    wt32_d = nc.declare_dram_parameter("wt32", [P, NW], f32, isOutput=False)
    wd0_d = nc.declare_dram_parameter("wd0", [P, KM * P], f16, isOutput=False)
    xw_d = nc.declare_dram_parameter("xw", [P, NCHUNK * TP], f16, isOutput=False)
    out_d = nc.declare_dram_parameter("out", [C, T], f16, isOutput=True)

    with TileContext(nc) as tc:
        with (
            tc.tile_pool(name="xpool", bufs=1) as xpool,
            tc.tile_pool(name="wpool", bufs=1) as wpool,
            tc.tile_pool(name="opool", bufs=4) as opool,
            tc.tile_pool(name="ppool", bufs=4, space="PSUM") as ppool,
        ):
            # PE warmup scratch: memset on GpSimd (earliest-free engine;
            # the Tile framework rejects reads of never-written tiles).
            # Small so the memset is ~180ns and warmup starts ASAP.
            scr = wpool.tile([P, P], f16, name="scr", tag="scr")
            nc.gpsimd.memset(scr[:, :], 0.0)

            # small weight DMAs on the Sync queue (its first output DMA
            # is much later); chunk0's diag stationary comes prebuilt
            # from the host so no GpSimd expansion gates the first matmul
            wcol = wpool.tile([P, NW], f16, tag="wcol")
            nc.sync.dma_start(out=wcol, in_=wt_d[:, :])
            wcol32 = wpool.tile([P, NW], f32, name="wcol32", tag="wcol32")
            nc.sync.dma_start(out=wcol32, in_=wt32_d[:, :])
            wd0 = wpool.tile([P, KM * P], f16, name="wd0", tag="wd0")
            nc.sync.dma_start(out=wd0, in_=wd0_d[:, :])

            # x loads get the Scalar DGE queue to themselves (ACT is idle
            # in this design); chunk0 split so pair-tile 0 (cols 0..1026)
            # lands sooner
            xts = []
            xt0 = xpool.tile([P, TP], f16, name="xt0", tag="xt0")
            h0 = TJ + K - 1  # 515: first half-tile + tap halo
            h1 = 2 * TJ + K - 1  # 1027: pair-tile 0
            h2 = 4 * TJ + K - 1  # 2051: pair-tile 1
            nc.scalar.dma_start(out=xt0[:, :h0], in_=xw_d[:, 0:h0])
            nc.scalar.dma_start(out=xt0[:, h0:h1], in_=xw_d[:, h0:h1])
            nc.scalar.dma_start(out=xt0[:, h1:h2], in_=xw_d[:, h1:h2])
            nc.scalar.dma_start(out=xt0[:, h2:], in_=xw_d[:, h2:TP])
            xts.append(xt0)
            for c in range(1, NCHUNK):
                xt = xpool.tile([P, TP], f16, name=f"xt{c}", tag=f"xt{c}")
                nc.scalar.dma_start(out=xt, in_=xw_d[:, c * TP : (c + 1) * TP])
                xts.append(xt)

            # expand wcol into per-chunk diag stationary tiles (taps 0..2)
            # on GpSimd for chunks 1..3 (chunk 0 is DMA'd from the host);
            # separate tiles so each chunk's matmuls wait only on its own
            wts = [wd0]
            for c in range(1, NCHUNK):
                wt = wpool.tile([P, KM * P], f16, name=f"wd{c}", tag=f"wd{c}")
                for k in range(KM):
                    idx = c * K + k
                    wsrc = bass.AP(
                        wcol.tensor, wcol.offset + idx, [[NW, P], [0, P]]
                    )
                    nc.gpsimd.affine_select(
                        out=wt[:, k * P : (k + 1) * P],
                        in_=wsrc,
                        compare_op=mybir.AluOpType.is_equal,
                        fill=0.0,
                        base=0,
                        # iota[p, i] = p - i; == 0 on the diagonal
                        pattern=[[-1, P]],
                        channel_multiplier=1,
                    )
                wts.append(wt)

            # PE p-state warmup: results land in a PSUM tile that the
            # real loop later overwrites. Groups of 3 accumulating
            # matmuls alternating between the tile's two banks, so they
            # issue back-to-back like the real taps do.
            ptw = ppool.tile([P, 2 * TJ], f32, name="ptw", tag="pt")
            for i in range(NWARM):
                half = (i // 3) % 2
                nc.tensor.matmul(
                    ptw[:, half * TJ : half * TJ + P],
                    scr[:, :P],
                    scr[:, :P],
                    start=(i % 3 == 0),
                    stop=(i % 3 == 2),
                )

            TJ2 = 2 * TJ  # j-tile pair: one 2-bank PSUM tile, one DVE pass
            for chunk in range(NCHUNK):
                xv = xts[chunk]
                wt = wts[chunk]
                w3 = wcol32[:, chunk * K + KM : chunk * K + KM + 1]  # [128,1]
                ot = opool.tile([P, T], f16, tag="ot")
                for jj in range(NJ // 2):
                    base = jj * TJ2
                    if (chunk, jj) in GP_OFF:
                        # whole 4-tap conv for this pair on GpSimd in
                        # SBUF: t = x0*w0; t = x_k*w_k + t; ot = x3*w3 + t
                        wk = lambda k: wcol32[:, chunk * K + k : chunk * K + k + 1]
                        nc.gpsimd.tensor_scalar_mul(
                            gpa[:, :], xv[:, base : base + TJ2], wk(0)
                        )
                        nc.gpsimd.scalar_tensor_tensor(
                            out=gpb[:, :],
                            in0=xv[:, base + 1 : base + 1 + TJ2],
                            scalar=wk(1),
                            in1=gpa[:, :],
                            op0=mybir.AluOpType.mult,
                            op1=mybir.AluOpType.add,
                        )
                        nc.gpsimd.scalar_tensor_tensor(
                            out=gpa[:, :],
                            in0=xv[:, base + 2 : base + 2 + TJ2],
                            scalar=wk(2),
                            in1=gpb[:, :],
                            op0=mybir.AluOpType.mult,
                            op1=mybir.AluOpType.add,
                        )
                        nc.gpsimd.scalar_tensor_tensor(
                            out=ot[:, base : base + TJ2],
                            in0=xv[:, base + K - 1 : base + K - 1 + TJ2],
                            scalar=w3,
                            in1=gpa[:, :],
                            op0=mybir.AluOpType.mult,
                            op1=mybir.AluOpType.add,
                        )
                        nc.sync.dma_start(
                            out=out_d[chunk * P : (chunk + 1) * P, base : base + TJ2],
                            in_=ot[:, base : base + TJ2],
                        )
                        continue
                    pt = ppool.tile([P, TJ2], f32, name="pt", tag="pt")
                    for half in range(2):
                        j = 2 * jj + half
                        for k in range(KM):
                            nc.tensor.matmul(
                                pt[:, half * TJ : (half + 1) * TJ],
                                wt[:, k * P : (k + 1) * P],
                                xv[:, j * TJ + k : j * TJ + k + TJ],
                                start=(k == 0),
                                stop=(k == KM - 1),
                            )
                    # fused tap-3 + downcast drain: ot = (x_s3 * w3) + pt
                    # The very last pair runs as two 512-wide passes with
                    # per-piece shipping to shorten the kernel tail.
                    last = chunk == NCHUNK - 1
                    tail = last and jj == NJ // 2 - 1
                    nsub = 2 if tail else 1
                    sub = TJ2 // nsub
                    for s in range(nsub):
                        lo = base + s * sub
                        nc.vector.scalar_tensor_tensor(
                            out=ot[:, lo : lo + sub],
                            in0=xv[:, lo + K - 1 : lo + K - 1 + sub],
                            scalar=w3,
                            in1=pt[:, s * sub : (s + 1) * sub],
                            op0=mybir.AluOpType.mult,
                            op1=mybir.AluOpType.add,
                        )
                        if last:
                            nc.sync.dma_start(
                                out=out_d[chunk * P : (chunk + 1) * P, lo : lo + sub],
                                in_=ot[:, lo : lo + sub],
                            )
                    # PE/DVE pairs of chunks 0..2 ship per half-chunk,
                    # except halves containing a GpSimd pair (those pairs
                    # ship on their own above) — ship the DVE pair alone.
                    if not last and (jj == NJ // 4 - 1 or jj == NJ // 2 - 1):
                        half_lo = 0 if jj < NJ // 4 else NJ // 4
                        pairs = [
                            p
                            for p in range(half_lo, half_lo + NJ // 4)
                            if (chunk, p) not in GP_OFF
                        ]
                        # contiguous runs of non-offloaded pairs
                        run_start = None
                        for p in pairs + [None]:
                            if run_start is None:
                                run_start = p
                                run_end = p
                            elif p is not None and p == run_end + 1:
                                run_end = p
                            else:
                                lo_c = run_start * TJ2
                                hi_c = (run_end + 1) * TJ2
                                nc.sync.dma_start(
                                    out=out_d[
                                        chunk * P : (chunk + 1) * P, lo_c:hi_c
                                    ],
                                    in_=ot[:, lo_c:hi_c],
                                )
                                run_start = p
                                run_end = p

    nc.compile()
    return nc


def _prep_inputs(x: np.ndarray, weight: np.ndarray):
    # wcol[p, chunk*K + k] = weight[chunk*P + p, k]
    wcol = np.ascontiguousarray(
        weight.reshape(NCHUNK, P, K).transpose(1, 0, 2).reshape(P, NW)
    ).astype(np.float16)
    # chunk0's diag stationary prebuilt: wd0[p, k*P + p] = weight[p, k]
    wd0 = np.zeros((P, KM * P), dtype=np.float16)
    for k in range(KM):
        wd0[np.arange(P), k * P + np.arange(P)] = weight[:P, k].astype(np.float16)
    xs = []
    for b in range(B):
        xp = np.zeros((C, TP), dtype=np.float32)
        xp[:, K - 1 :] = x[b].T  # [512, 4099], 3 leading zeros
        xw = np.ascontiguousarray(
            xp.reshape(NCHUNK, P, TP).transpose(1, 0, 2).reshape(P, NCHUNK * TP)
        ).astype(np.float16)
        xs.append(xw)
    wcol32 = np.ascontiguousarray(
        weight.reshape(NCHUNK, P, K).transpose(1, 0, 2).reshape(P, NW)
    ).astype(np.float32)
    return xs, wcol, wd0, wcol32


def _ensure_axon_hooks():
    """This image's antenv package lacks axon_hooks; synthesize it so a
    trace=True / BASS_TRACE run of run_bass_kernel_spmd can profile
    instead of crashing on import."""
    import sys
    import types

    if "antenv.axon_hooks" in sys.modules:
        return
    mod = types.ModuleType("antenv.axon_hooks")
    state = {"hook": None}
    mod.set_axon_ntff_profile_hook = lambda h: state.__setitem__("hook", h)
    mod.get_axon_ntff_profile_hook = lambda: state["hook"]
    sys.modules["antenv.axon_hooks"] = mod
    try:
        if "/root/.axon_site" not in sys.path:
            sys.path.insert(0, "/root/.axon_site")
        from trn_agent_boot.trn_boot import _ntff_profile_via_ctypes

        mod.set_axon_ntff_profile_hook(
            _ntff_profile_via_ctypes("/opt/axon/libaxon_pjrt.so")
        )
    except Exception:
        pass  # hook stays None; concourse degrades to no-trace


def kernel(x: np.ndarray, weight: np.ndarray) -> np.ndarray:
    global _compiled
    _ensure_axon_hooks()
    from concourse import bass_utils

    x = np.ascontiguousarray(x, dtype=np.float32)
    weight = np.ascontiguousarray(weight, dtype=np.float32)

    if _compiled is None:
        _compiled = _build()
    nc = _compiled

    xs, wcol, wd0, wcol32 = _prep_inputs(x, weight)
    in_maps = [
        {"xw": xs[b], "wt": wcol, "wd0": wd0, "wt32": wcol32} for b in range(B)
    ]
    res = bass_utils.run_bass_kernel_spmd(nc, in_maps, core_ids=list(range(B)))

    out = np.empty((B, T, C), dtype=np.float32)
    for b in range(B):
        out[b] = np.asarray(res.results[b]["out"]).astype(np.float32).T
    return out
